# revision 1
# baseline (speedup 1.0000x reference)
"""COO SpMM (GNN message passing) on 8 Trainium2 NeuronCores.

out[b, d] = sum_e vals[e] * x[b, col[e]]  for row[e]==d,  + bias[d]

Strategy (dst-sharded):
  - Core k owns destination rows [k*12500, (k+1)*12500).
  - Host: per core, sort dsts by degree (desc), tile into 98 blocks of 128
    dsts, pad each block's edge lists to the block max degree, and emit a
    slot-interleaved gather schedule: group g holds slot-j edges of all 128
    dsts of one block (partition p <-> block dst p).
  - Device: for each group, indirect-DMA-gather the 128 referenced x
    columns (pre-transposed xT rows) into a [128, 64] tile, scale rows by
    the edge vals (per-partition scalar), and accumulate into the block's
    PSUM tile via an identity matmul.  Copy out with bias added.
  - Host: un-permute rows, concat cores, transpose back to [64, 100000].
"""
import sys
import numpy as np

sys.path.insert(0, "/opt/trn_rl_repo")

NUM_SRC = 100000
NUM_DST = 100000
NNZ = 3200000
BATCH = 64
NCORES = 8
DSH = NUM_DST // NCORES          # 12500 dsts per core
P = 128
NBLK = (DSH + P - 1) // P        # 98 blocks
DPAD = NBLK * P                  # 12544


def _preprocess(row, col, vals):
    """Build per-core gather schedules.

    Returns (ng_list, per_core) where ng_list[b] = group count of block b
    (shared across cores) and per_core[k] = (offs, valsw, perm) with
    offs/valsw shaped [128, NGROUPS] (slot-partition-major).
    """
    row = np.asarray(row).astype(np.int64)
    col = np.asarray(col).astype(np.int64)
    vals = np.asarray(vals).astype(np.float32)

    cores = []
    deg_blocks = np.zeros((NCORES, NBLK), dtype=np.int64)
    for k in range(NCORES):
        m = (row >= k * DSH) & (row < (k + 1) * DSH)
        r = row[m] - k * DSH
        c = col[m]
        v = vals[m]
        deg = np.bincount(r, minlength=DPAD)
        perm = np.argsort(-deg, kind="stable")          # dstrank -> local dst
        rankof = np.empty(DPAD, dtype=np.int64)
        rankof[perm] = np.arange(DPAD)
        dstrank = rankof[r]
        order = np.argsort(dstrank, kind="stable")
        dstrank_s = dstrank[order]
        c_s = c[order]
        v_s = v[order]
        # j = index of edge within its dst
        counts = np.bincount(dstrank_s, minlength=DPAD)
        starts = np.concatenate([[0], np.cumsum(counts)[:-1]])
        j = np.arange(len(dstrank_s)) - starts[dstrank_s]
        blk = dstrank_s // P
        slot = dstrank_s % P
        deg_blocks[k] = np.maximum.reduceat(
            deg[perm], np.arange(0, DPAD, P)) if len(deg) else 0
        cores.append((blk, slot, j, c_s, v_s, perm))

    ng = deg_blocks.max(axis=0)
    ng = np.maximum(ng, 1)                              # >=1 group per block
    gbase = np.concatenate([[0], np.cumsum(ng)[:-1]])
    ngroups = int(ng.sum())

    per_core = []
    for k in range(NCORES):
        blk, slot, j, c_s, v_s, perm = cores[k]
        offs = np.zeros((P, ngroups), dtype=np.int32)
        valsw = np.zeros((P, ngroups), dtype=np.float32)
        g = gbase[blk] + j
        offs[slot, g] = c_s.astype(np.int32)
        valsw[slot, g] = v_s
        per_core.append((offs, valsw, perm))
    return ng, gbase, ngroups, per_core


_BUILD_CACHE = {}


def _build(ngroups, ng):
    key = (ngroups, tuple(ng))
    if key in _BUILD_CACHE:
        return _BUILD_CACHE[key]
    import concourse.bacc as bacc
    import concourse.bass as bass
    import concourse.mybir as mybir
    import concourse.tile as tile
    from concourse.masks import make_identity

    nc = bacc.Bacc("TRN2", target_bir_lowering=False, debug=False,
                   num_devices=NCORES)
    xT = nc.dram_tensor("xT", [NUM_SRC, BATCH], mybir.dt.float32,
                        kind="ExternalInput")
    offs_d = nc.dram_tensor("offs", [P, ngroups], mybir.dt.int32,
                            kind="ExternalInput")
    valsw_d = nc.dram_tensor("valsw", [P, ngroups], mybir.dt.float32,
                             kind="ExternalInput")
    biasp_d = nc.dram_tensor("biasp", [P, NBLK], mybir.dt.float32,
                             kind="ExternalInput")
    out_d = nc.dram_tensor("out", [NBLK, P, BATCH], mybir.dt.float32,
                           kind="ExternalOutput")

    with tile.TileContext(nc) as tc:
        with (
            tc.tile_pool(name="const", bufs=1) as const_pool,
            tc.tile_pool(name="gat", bufs=16) as gat_pool,
            tc.tile_pool(name="scaled", bufs=8) as sc_pool,
            tc.tile_pool(name="psum", bufs=4, space="PSUM") as psum_pool,
            tc.tile_pool(name="outp", bufs=4) as out_pool,
        ):
            ident = const_pool.tile([P, P], mybir.dt.float32)
            make_identity(nc, ident[:])
            offs_t = const_pool.tile([P, ngroups], mybir.dt.int32)
            nc.sync.dma_start(offs_t[:], offs_d[:])
            valsw_t = const_pool.tile([P, ngroups], mybir.dt.float32)
            nc.sync.dma_start(valsw_t[:], valsw_d[:])
            biasp_t = const_pool.tile([P, NBLK], mybir.dt.float32)
            nc.sync.dma_start(biasp_t[:], biasp_d[:])

            g = 0
            for b in range(NBLK):
                ps = psum_pool.tile([P, BATCH], mybir.dt.float32, space="PSUM")
                nblocks_b = int(ng[b])
                for j in range(nblocks_b):
                    gt = gat_pool.tile([P, BATCH], mybir.dt.float32)
                    nc.gpsimd.indirect_dma_start(
                        out=gt[:], out_offset=None, in_=xT[:],
                        in_offset=bass.IndirectOffsetOnAxis(
                            ap=offs_t[:, g:g + 1], axis=0),
                    )
                    st = sc_pool.tile([P, BATCH], mybir.dt.float32)
                    nc.vector.tensor_scalar(
                        out=st[:], in0=gt[:], scalar1=valsw_t[:, g:g + 1],
                        scalar2=None, op0=mybir.AluOpType.mult)
                    nc.tensor.matmul(ps[:], ident[:], st[:],
                                     start=(j == 0), stop=(j == nblocks_b - 1))
                    g += 1
                ot = out_pool.tile([P, BATCH], mybir.dt.float32)
                nc.vector.tensor_scalar(
                    out=ot[:], in0=ps[:], scalar1=biasp_t[:, b:b + 1],
                    scalar2=None, op0=mybir.AluOpType.add)
                nc.sync.dma_start(out_d[b], ot[:])
    nc.compile()
    _BUILD_CACHE[key] = nc
    return nc


def kernel(x, vals, bias, row, col):
    from concourse.bass_utils import run_bass_kernel_spmd

    x = np.asarray(x)
    bias = np.asarray(bias).astype(np.float32)
    ng, gbase, ngroups, per_core = _preprocess(row, col, vals)
    nc = _build(ngroups, ng)

    xTv = np.ascontiguousarray(np.asarray(x).astype(np.float32).T)  # [100000, 64]
    in_maps = []
    for k in range(NCORES):
        offs, valsw, perm = per_core[k]
        biasp = np.zeros((P, NBLK), dtype=np.float32)
        dsts = perm.reshape(NBLK, P)                   # [b, p] -> local dst
        valid = dsts < DSH
        bp = np.zeros((NBLK, P), dtype=np.float32)
        bp[valid] = bias[k * DSH:(k + 1) * DSH][dsts[valid]]
        biasp = bp.T.copy()                            # [P, NBLK]
        in_maps.append({"xT": xTv, "offs": offs, "valsw": valsw,
                        "biasp": biasp})

    res = run_bass_kernel_spmd(nc, in_maps, list(range(NCORES)))

    out = np.empty((NUM_DST, BATCH), dtype=np.float32)
    for k in range(NCORES):
        rows = res.results[k]["out"].reshape(DPAD, BATCH)
        local = np.zeros((DPAD, BATCH), dtype=np.float32)
        local[per_core[k][2]] = rows
        out[k * DSH:(k + 1) * DSH] = local[:DSH]
    return np.ascontiguousarray(out.T)


# revision 3
# speedup vs baseline: 1341.1718x; 1341.1718x over previous
"""COO SpMM (GNN message passing) on 8 Trainium2 NeuronCores.

out[b, d] = sum_e vals[e] * x[b, col[e]]  for row[e]==d,  + bias[d]

Strategy (dst-sharded):
  - Core k owns destination rows [k*12500, (k+1)*12500).
  - Host: per core, sort dsts by degree (desc), tile into 98 blocks of 128
    dsts, pad each block's edge lists to the block max degree, and emit a
    slot-interleaved gather schedule: group g holds slot-j edges of all 128
    dsts of one block (partition p <-> block dst p).
  - Device: gather calls fetch 8 groups at a time (1024 edges) from a
    512B-pitch transposed-x table via indirect DMA; each group tile is
    scaled by edge vals (per-partition scalar) and accumulated into the
    block's PSUM tile via an identity matmul.  Copy out with bias added.
  - Host: un-permute rows, concat cores, transpose back to [64, 100000].
"""
import sys
import numpy as np

sys.path.insert(0, "/opt/trn_rl_repo")

NUM_SRC = 100000
NUM_DST = 100000
NNZ = 3200000
BATCH = 64
NCORES = 8
DSH = NUM_DST // NCORES          # 12500 dsts per core
P = 128
NBLK = (DSH + P - 1) // P        # 98 blocks
DPAD = NBLK * P                  # 12544
GPC = 1                          # groups per gather call
ROWPAD = 64                      # xT2 row = 64 f32 = 256B


def _preprocess(row, col, vals):
    row = np.asarray(row).astype(np.int64)
    col = np.asarray(col).astype(np.int64)
    vals = np.asarray(vals).astype(np.float32)

    cores = []
    deg_blocks = np.zeros((NCORES, NBLK), dtype=np.int64)
    for k in range(NCORES):
        m = (row >= k * DSH) & (row < (k + 1) * DSH)
        r = row[m] - k * DSH
        c = col[m]
        v = vals[m]
        deg = np.bincount(r, minlength=DPAD)
        perm = np.argsort(-deg, kind="stable")          # dstrank -> local dst
        rankof = np.empty(DPAD, dtype=np.int64)
        rankof[perm] = np.arange(DPAD)
        dstrank = rankof[r]
        order = np.argsort(dstrank, kind="stable")
        dstrank_s = dstrank[order]
        counts = np.bincount(dstrank_s, minlength=DPAD)
        starts = np.concatenate([[0], np.cumsum(counts)[:-1]])
        j = np.arange(len(dstrank_s)) - starts[dstrank_s]
        cores.append((dstrank_s // P, dstrank_s % P, j, c[order], v[order], perm))
        deg_blocks[k] = np.maximum.reduceat(deg[perm], np.arange(0, DPAD, P))

    ng = deg_blocks.max(axis=0)
    ng = np.maximum(ng, 1)
    ng[-1] += (-int(ng.sum())) % GPC                    # pad to call multiple
    gbase = np.concatenate([[0], np.cumsum(ng)[:-1]])
    ngroups = int(ng.sum())

    per_core = []
    for k in range(NCORES):
        blk, slot, j, c_s, v_s, perm = cores[k]
        offs = np.zeros((P, ngroups), dtype=np.int32)
        valsw = np.zeros((P, ngroups), dtype=np.float32)
        g = gbase[blk] + j
        offs[slot, g] = c_s.astype(np.int32)
        valsw[slot, g] = v_s
        per_core.append((offs, valsw, perm))
    return ng, gbase, ngroups, per_core


_BUILD_CACHE = {}


def _build(ngroups, ng):
    key = (ngroups, tuple(int(v) for v in ng))
    if key in _BUILD_CACHE:
        return _BUILD_CACHE[key]
    import concourse.bacc as bacc
    import concourse.bass as bass
    import concourse.mybir as mybir
    import concourse.tile as tile
    from concourse.masks import make_identity

    nc = bacc.Bacc("TRN2", target_bir_lowering=False, debug=False,
                   num_devices=NCORES, dynamic_dma_scratch_size=65536)
    xT2 = nc.dram_tensor("xT2", [NUM_SRC, ROWPAD], mybir.dt.float32,
                         kind="ExternalInput")
    offs_d = nc.dram_tensor("offs", [P, ngroups], mybir.dt.int32,
                            kind="ExternalInput")
    valsw_d = nc.dram_tensor("valsw", [P, ngroups], mybir.dt.float32,
                             kind="ExternalInput")
    biasp_d = nc.dram_tensor("biasp", [P, NBLK], mybir.dt.float32,
                             kind="ExternalInput")
    out_d = nc.dram_tensor("out", [NBLK, P, BATCH], mybir.dt.float32,
                           kind="ExternalOutput")

    blk_of_g = np.repeat(np.arange(NBLK), ng)
    jpos = np.concatenate([np.arange(n) for n in ng])
    nlast = {b: int(n) - 1 for b, n in enumerate(ng)}

    with tile.TileContext(nc) as tc:
        with (
            tc.tile_pool(name="const", bufs=1) as const_pool,
            tc.tile_pool(name="gat", bufs=32) as gat_pool,
            tc.tile_pool(name="scaled", bufs=16) as sc_pool,
            tc.tile_pool(name="psum", bufs=8, space="PSUM") as psum_pool,
            tc.tile_pool(name="outp", bufs=4) as out_pool,
        ):
            ident = const_pool.tile([P, P], mybir.dt.float32)
            make_identity(nc, ident[:])
            offs_t = const_pool.tile([P, ngroups], mybir.dt.int32)
            nc.sync.dma_start(offs_t[:], offs_d[:])
            valsw_t = const_pool.tile([P, ngroups], mybir.dt.float32)
            nc.sync.dma_start(valsw_t[:], valsw_d[:])
            biasp_t = const_pool.tile([P, NBLK], mybir.dt.float32)
            nc.sync.dma_start(biasp_t[:], biasp_d[:])

            ps = None
            gt = None
            for g in range(ngroups):
                gt = gat_pool.tile([P, ROWPAD], mybir.dt.float32)
                nc.gpsimd.indirect_dma_start(
                    out=gt[:], out_offset=None, in_=xT2[:],
                    in_offset=bass.IndirectOffsetOnAxis(
                        ap=offs_t[:, g:g + 1], axis=0),
                )
                b = int(blk_of_g[g])
                j = int(jpos[g])
                if j == 0:
                    ps = psum_pool.tile([P, BATCH], mybir.dt.float32,
                                        space="PSUM")
                st = sc_pool.tile([P, BATCH], mybir.dt.float32)
                nc.vector.tensor_scalar(
                    out=st[:], in0=gt[:],
                    scalar1=valsw_t[:, g:g + 1],
                    scalar2=None, op0=mybir.AluOpType.mult)
                nc.tensor.matmul(ps[:], ident[:], st[:],
                                 start=(j == 0), stop=(j == nlast[b]))
                if j == nlast[b]:
                    ot = out_pool.tile([P, BATCH], mybir.dt.float32)
                    nc.vector.tensor_scalar(
                        out=ot[:], in0=ps[:], scalar1=biasp_t[:, b:b + 1],
                        scalar2=None, op0=mybir.AluOpType.add)
                    nc.sync.dma_start(out_d[b], ot[:])
    nc.compile()
    _BUILD_CACHE[key] = nc
    return nc


def kernel(x, vals, bias, row, col):
    from concourse.bass_utils import run_bass_kernel_spmd

    x = np.asarray(x)
    bias = np.asarray(bias).astype(np.float32)
    ng, gbase, ngroups, per_core = _preprocess(row, col, vals)
    nc = _build(ngroups, ng)

    xT2v = np.zeros((NUM_SRC, ROWPAD), dtype=np.float32)
    xT2v[:, :BATCH] = np.asarray(x).astype(np.float32).T
    in_maps = []
    for k in range(NCORES):
        offs, valsw, perm = per_core[k]
        dsts = perm.reshape(NBLK, P)
        valid = dsts < DSH
        bp = np.zeros((NBLK, P), dtype=np.float32)
        bp[valid] = bias[k * DSH:(k + 1) * DSH][dsts[valid]]
        in_maps.append({"xT2": xT2v, "offs": offs, "valsw": valsw,
                        "biasp": bp.T.copy()})

    res = run_bass_kernel_spmd(nc, in_maps, list(range(NCORES)))

    out = np.empty((NUM_DST, BATCH), dtype=np.float32)
    for k in range(NCORES):
        rows = res.results[k]["out"].reshape(DPAD, BATCH)
        local = np.zeros((DPAD, BATCH), dtype=np.float32)
        local[per_core[k][2]] = rows
        out[k * DSH:(k + 1) * DSH] = local[:DSH]
    return np.ascontiguousarray(out.T)
